# revision 43
# baseline (speedup 1.0000x reference)
"""SAM-style windowed-attention transformer block on 8 Trainium2 cores.

Transfer-optimized. The axon-tunneled dispatch is wire-bound
(~50-100 MB/s; device compute is ~ms), so the design minimizes bytes
shipped per dispatch (~59 MB vs ~690 MB for the fp32 baseline):

- x is shipped as int8 with one dynamic scale (LN is scale-invariant,
  so quantization noise only enters through the attention/MLP deltas);
  the residual is re-added in f32 on the host.
- The four big weight matrices are shipped as 1/8 row-shards (one per
  core) and reassembled on device with NeuronLink AllGather collectives;
  attention weights travel in bf16, MLP weights in int8 with per-row
  scales (dequantized once to bf16 in DRAM after the gather).
- The decomposed rel-pos bias (previously a 43-GFLOP host matmul plus an
  18 MB ship) is computed on device from q with 448 small PE matmuls
  against tiny (128,14,14) tables.
- The device returns delta = out - x as int8 with per-feature scales
  (abs-max reduced on device); the host dequantizes and adds x in f32.

Compute layout is unchanged from v1: data-parallel over the 104 padded
attention windows (13 per core), feature-on-partition "T layout", LN
reductions and softmax normalization via ones-matmuls on the PE,
rel-pos bias injected into the logits PSUM via one-hot matmuls.
"""

import sys

sys.path.insert(0, "/opt/trn_rl_repo")

import numpy as np
import ml_dtypes

BF = ml_dtypes.bfloat16

DIM = 1024
NH = 16
HD = 64
WS = 14
DFF = 4096
EPS = 1e-6
B, H, W = 4, 64, 64
T = WS * WS          # 196 tokens / window
NWIN = 100           # real windows
NWINP = 104          # padded to 8*13
WPC = NWINP // 8     # 13 windows per core
TOK = WPC * T        # 2548
TOKP = 2560          # padded to 5*512
P = 128
KD = DIM // P        # 8
NT = TOKP // 512     # 5

_CACHE = {}

CW = 2080             # compact real-token columns per core (balanced windows)

# per-slot geometry: (slot, compact col offset, real rows, real cols)
# slots 0-7 full 14x14, 8-9 bottom-edge 8x14, 10-11 right-edge 14x8,
# 12 corner 8x8 (cores 4-7 ship zeros there = pad windows)
_SLOT_GEOM = (
    [(s, 196 * s, 14, 14) for s in range(8)]
    + [(8 + e, 1568 + 112 * e, 8, 14) for e in range(2)]
    + [(10 + e, 1792 + 112 * e, 14, 8) for e in range(2)]
    + [(12, 2016, 8, 8)]
)


def _win_assign():
    fulls = [(b, i, j) for b in range(B) for i in range(4) for j in range(4)]
    bots = [(b, 4, j) for b in range(B) for j in range(4)]
    rights = [(b, i, 4) for b in range(B) for i in range(4)]
    corners = [(b, 4, 4) for b in range(B)]
    return [fulls[8 * c:8 * c + 8] + bots[2 * c:2 * c + 2]
            + rights[2 * c:2 * c + 2] + [corners[c] if c < 4 else None]
            for c in range(8)]


_WIN_ASSIGN = _win_assign()

# single-blob input packing: one param per core instead of 18 (each extra
# array adds per-dispatch transfer overhead on the axon tunnel)
_BLOB_SPEC = [
    ("x8", (DIM, CW), "i1"),
    ("wqkv_sh", (P, 3 * DIM), "i1"),
    ("wproj_sh", (P, DIM), "i1"),
    ("w1_sh", (P, DFF), "i1"),
    ("w2_sh", (512, DIM), "i1"),
    ("xscl", (P, 1), "f4"),
    ("rs_w1", (DIM, 1), "f4"),
    ("rs_w2", (DFF, 1), "f4"),
    ("rs_qkv", (3 * DIM, 1), "f4"),
    ("rs_pj", (DIM, 1), "f4"),
    ("bqkv", (3 * DIM, 1), "f4"),
    ("bproj", (DIM, 1), "f4"),
    ("b1", (DFF, 1), "f4"),
    ("b2", (DIM, 1), "f4"),
    ("khmat", (WS, T), "b2"),
    ("kwmat", (WS, T), "b2"),
    ("rhT8", (P, WS, WS), "b2"),
    ("rwT8", (P, WS, WS), "b2"),
]
_ESZ = {"i1": 1, "f4": 4, "b2": 2}


def _blob_offsets():
    offs, off = {}, 0
    for name, shape, dt in _BLOB_SPEC:
        n = int(np.prod(shape)) * _ESZ[dt]
        offs[name] = off
        off += (n + 511) // 512 * 512
    return offs, off


_BLOB_OFFS, _BLOB_NB = _blob_offsets()


def _pack_blob(mc):
    blob = np.zeros(_BLOB_NB, np.int8)
    for name, shape, dt in _BLOB_SPEC:
        npdt = {"i1": np.int8, "f4": np.float32, "b2": BF}[dt]
        a = np.ascontiguousarray(np.asarray(mc[name], npdt).reshape(shape))
        raw = np.frombuffer(a.tobytes(), np.int8)
        blob[_BLOB_OFFS[name]:_BLOB_OFFS[name] + raw.size] = raw
    return blob


def _hostprep(x, norm1_scale, norm1_bias, qkv_kernel, qkv_bias, rel_pos_h,
              rel_pos_w, proj_kernel, proj_bias, norm2_scale, norm2_bias,
              fc1_kernel, fc1_bias, fc2_kernel, fc2_bias):
    f = np.float32
    x = np.asarray(x, f)

    # LN affine folded into qkv / fc1 weights; HD^-0.5 folded into the K
    # block (so q and the rel-pos tables stay unscaled and the int8
    # per-row scales absorb the factor exactly)
    wqkv = (np.asarray(norm1_scale, f)[:, None] * np.asarray(qkv_kernel, f))
    bqkv = (np.asarray(norm1_bias, f) @ np.asarray(qkv_kernel, f)
            + np.asarray(qkv_bias, f))
    sc = np.float32(HD ** -0.5)
    wqkv = wqkv.copy()
    wqkv[:, DIM:2 * DIM] *= sc
    bqkv = bqkv.copy()
    bqkv[DIM:2 * DIM] *= sc
    w1 = (np.asarray(norm2_scale, f)[:, None] * np.asarray(fc1_kernel, f))
    b1 = (np.asarray(norm2_bias, f) @ np.asarray(fc1_kernel, f)
          + np.asarray(fc1_bias, f))

    # rel-pos tables, feature (c) on partitions, replicated to 128
    # partitions so lhsT can address either 64-partition half.
    coords = (np.arange(WS)[:, None] - np.arange(WS)[None, :] + WS - 1)
    rh = np.asarray(rel_pos_h, f)[coords]   # (14q, 14k, 64)
    rw = np.asarray(rel_pos_w, f)[coords]
    rhT8 = np.ascontiguousarray(rh.transpose(2, 0, 1))  # (64,14,14)
    rwT8 = np.ascontiguousarray(rw.transpose(2, 0, 1))
    rhT8 = np.concatenate([rhT8, rhT8], axis=0).astype(BF)     # (128,14,14)
    rwT8 = np.concatenate([rwT8, rwT8], axis=0).astype(BF)

    s = np.arange(T)
    khmat = (s[None, :] // WS == np.arange(WS)[:, None]).astype(BF)
    kwmat = (s[None, :] % WS == np.arange(WS)[:, None]).astype(BF)

    def quant_rows(wm):
        rs = np.maximum(np.abs(wm).max(axis=1, keepdims=True), 1e-30) / 127.0
        w8 = np.clip(np.rint(wm / rs), -127, 127).astype(np.int8)
        return w8, rs.astype(f)

    # qkv quantized per q/k/v block (per-row scales within each block)
    qkv_blocks = [quant_rows(wqkv[:, b * DIM:(b + 1) * DIM]) for b in range(3)]
    wqkv8 = np.concatenate([b[0] for b in qkv_blocks], axis=1)
    rs_qkv = np.concatenate([b[1] for b in qkv_blocks], axis=0)  # (3*DIM, 1)
    wproj8, rs_pj = quant_rows(np.asarray(proj_kernel, f))
    w18, rs_w1 = quant_rows(w1)
    w28, rs_w2 = quant_rows(np.asarray(fc2_kernel, f))

    common = {
        "bqkv": np.ascontiguousarray(bqkv[:, None]),
        "bproj": np.ascontiguousarray(np.asarray(proj_bias, f)[:, None]),
        "b1": np.ascontiguousarray(b1[:, None]),
        "b2": np.ascontiguousarray(np.asarray(fc2_bias, f)[:, None]),
        "khmat": khmat, "kwmat": kwmat,
        "rhT8": rhT8, "rwT8": rwT8,
        "rs_w1": rs_w1, "rs_w2": rs_w2,
        "rs_qkv": rs_qkv, "rs_pj": rs_pj,
    }
    # dynamic int8 quantization of x (LN makes the scale nearly free of
    # precision impact; the residual is re-added in f32 on the host)
    xscale = np.float32(max(float(np.abs(x).max()), 1e-30) / 127.0)
    xsclP = np.full((P, 1), xscale, np.float32)
    common["xscl"] = xsclP
    in_maps = []
    for c in range(8):
        mc = dict(common)
        xc = np.zeros((DIM, CW), np.int8)
        for sl, off, ri, rj in _SLOT_GEOM:
            wspec = _WIN_ASSIGN[c][sl]
            if wspec is None:
                continue
            b_, wi, wj = wspec
            blk = x[b_, 14 * wi:14 * wi + ri, 14 * wj:14 * wj + rj, :]
            blk = blk.reshape(ri * rj, DIM).T
            xc[:, off:off + ri * rj] = np.clip(
                np.rint(blk / xscale), -127, 127).astype(np.int8)
        mc["x8"] = xc
        mc["wqkv_sh"] = np.ascontiguousarray(wqkv8[c * P:(c + 1) * P])
        mc["wproj_sh"] = np.ascontiguousarray(wproj8[c * P:(c + 1) * P])
        mc["w1_sh"] = np.ascontiguousarray(w18[c * P:(c + 1) * P])
        mc["w2_sh"] = np.ascontiguousarray(w28[c * 512:(c + 1) * 512])
        in_maps.append({"blob": _pack_blob(mc)})
    return in_maps


def _build(io_dummy=False):
    import concourse.bass as bass
    import concourse.mybir as mybir
    import concourse.tile as tile
    from concourse import bacc
    from concourse.bass import ts

    f32 = mybir.dt.float32
    f32r = mybir.dt.float32r
    bf16 = mybir.dt.bfloat16
    AF = mybir.ActivationFunctionType
    r = lambda ap_: ap_.bitcast(f32r)

    nc = bacc.Bacc("TRN2", target_bir_lowering=False, debug=False,
                   num_devices=8)

    if io_dummy:
        # exec-isolation mode: params/outputs become DRAM scratch so the
        # dispatch ships ~no bytes; timing = fixed overhead + device exec
        def param(name, shape, dt, isOutput=False):
            return nc.dram_tensor("dmy_" + name, shape, dt)
        nc.declare_dram_parameter("dummy_in", [1, 1], f32, isOutput=False)
        nc.declare_dram_parameter("dummy_out", [1, 1], f32, isOutput=True)
    else:
        param = nc.declare_dram_parameter

    i8 = mybir.dt.int8
    blob_d = param("blob", [_BLOB_NB], i8, isOutput=False).ap()

    def bview(name, dt):
        shape = dict((n, s) for n, s, _ in _BLOB_SPEC)[name]
        esz = _ESZ[dict((n, d) for n, s, d in ((a, b, c) for a, b, c in _BLOB_SPEC))[name]]
        off = _BLOB_OFFS[name]
        n = int(np.prod(shape)) * esz
        flat = blob_d[off:off + n]
        if len(shape) == 2:
            ap_ = flat.rearrange("(r c) -> r c", r=shape[0])
        else:
            ap_ = flat.rearrange("(r a b) -> r a b", r=shape[0], a=shape[1])
        return ap_.bitcast(dt)

    x8_d = bview("x8", i8)
    xscl_d = bview("xscl", f32)
    wqkv_sh_d = bview("wqkv_sh", i8)
    wproj_sh_d = bview("wproj_sh", i8)
    w1_sh_d = bview("w1_sh", i8)
    w2_sh_d = bview("w2_sh", i8)
    rs1_d = bview("rs_w1", f32)
    rs2_d = bview("rs_w2", f32)
    rsq_d = bview("rs_qkv", f32)
    rspj_d = bview("rs_pj", f32)
    bqkv_d = bview("bqkv", f32)
    bproj_d = bview("bproj", f32)
    b1_d = bview("b1", f32)
    b2_d = bview("b2", f32)
    rhT8_d = bview("rhT8", bf16)
    rwT8_d = bview("rwT8", bf16)
    khm_d = bview("khmat", bf16)
    kwm_d = bview("kwmat", bf16)
    outT_d = param("outT", [DIM, CW], mybir.dt.int8, isOutput=True).ap()
    oscl_d = param("oscl", [DIM, 1], f32, isOutput=True).ap()

    # DRAM scratch
    w1_g8 = nc.dram_tensor("w1_g8", [DIM, DFF], i8).ap()
    w2_g8 = nc.dram_tensor("w2_g8", [DFF, DIM], i8).ap()
    wqkv_g = nc.dram_tensor("wqkv_g", [DIM, 3 * DIM], bf16).ap()
    wproj_g = nc.dram_tensor("wproj_g", [DIM, DIM], bf16).ap()
    w1_g = nc.dram_tensor("w1_g", [DIM, DFF], bf16).ap()
    w2_g = nc.dram_tensor("w2_g", [DFF, DIM], bf16).ap()
    wqkv_b = nc.dram_tensor("wqkv_b", [P, 3 * DIM], i8).ap()
    wproj_b = nc.dram_tensor("wproj_b", [P, DIM], i8).ap()
    w1_b = nc.dram_tensor("w1_b", [P, DFF], i8).ap()
    w2_b = nc.dram_tensor("w2_b", [512, DIM], i8).ap()
    wqkv_g8 = nc.dram_tensor("wqkv_g8", [DIM, 3 * DIM], i8).ap()
    wproj_g8 = nc.dram_tensor("wproj_g8", [DIM, DIM], i8).ap()

    qk_scr = nc.dram_tensor("qk_scr", [2 * DIM, TOKP], bf16).ap()
    v_scr = nc.dram_tensor("v_scr", [TOKP, DIM], bf16).ap()
    attn_scr = nc.dram_tensor("attn_scr", [DIM, TOKP], bf16).ap()
    dlt_scr = nc.dram_tensor("dlt_scr", [DIM, TOKP], bf16).ap()
    ln_scr = nc.dram_tensor("ln_scr", [2, TOKP], f32).ap()
    rs_scr = nc.dram_tensor("rs_scr", [NH, T], f32).ap()
    # rel bias scratch: relh [ki, head, window, qi, qj]; relw [kj, head,
    # window, qj, qi] (j-major so the per-j stores stay DMA-contiguous)
    relh_scr = nc.dram_tensor("relh_scr", [WS, NH, WPC, WS, WS], bf16).ap()
    relw_scr = nc.dram_tensor("relw_scr", [WS, NH, WPC, WS, WS], bf16).ap()

    RG = [list(range(8))]

    with tile.TileContext(nc) as tc:
        # ---- weight reassembly: shard -> bounce -> AllGather over NeuronLink
        nc.gpsimd.dma_start(out=wqkv_b[:], in_=wqkv_sh_d[:])
        nc.gpsimd.dma_start(out=wproj_b[:], in_=wproj_sh_d[:])
        nc.gpsimd.dma_start(out=w1_b[:], in_=w1_sh_d[:])
        nc.gpsimd.dma_start(out=w2_b[:], in_=w2_sh_d[:])
        nc.gpsimd.collective_compute("AllGather", mybir.AluOpType.bypass,
                                     replica_groups=RG, ins=[wqkv_b[:].opt()],
                                     outs=[wqkv_g8[:].opt()])
        nc.gpsimd.collective_compute("AllGather", mybir.AluOpType.bypass,
                                     replica_groups=RG, ins=[wproj_b[:].opt()],
                                     outs=[wproj_g8[:].opt()])
        nc.gpsimd.collective_compute("AllGather", mybir.AluOpType.bypass,
                                     replica_groups=RG, ins=[w1_b[:].opt()],
                                     outs=[w1_g8[:].opt()])
        nc.gpsimd.collective_compute("AllGather", mybir.AluOpType.bypass,
                                     replica_groups=RG, ins=[w2_b[:].opt()],
                                     outs=[w2_g8[:].opt()])

        # ---- dequantize gathered int8 weights to bf16 in DRAM (one pass)
        with tc.tile_pool(name="dq", bufs=8) as dqp, \
             tc.tile_pool(name="dqs", bufs=2) as dqsp:
            for g8, gf, rs_d, rs_off, R, C, c0 in (
                    (w1_g8, w1_g, rs1_d, 0, DIM, DFF, 0),
                    (w2_g8, w2_g, rs2_d, 0, DFF, DIM, 0),
                    (wqkv_g8, wqkv_g, rsq_d, 0, DIM, DIM, 0),
                    (wqkv_g8, wqkv_g, rsq_d, DIM, DIM, DIM, DIM),
                    (wqkv_g8, wqkv_g, rsq_d, 2 * DIM, DIM, DIM, 2 * DIM),
                    (wproj_g8, wproj_g, rspj_d, 0, DIM, DIM, 0)):
                for rt in range(R // P):
                    sclw = dqsp.tile([P, 1], f32, tag="sclw", name="sclw")
                    nc.sync.dma_start(out=sclw[:], in_=rs_d[rs_off + rt * P:rs_off + (rt + 1) * P, :])
                    for ct in range(C // 512):
                        wt8 = dqp.tile([P, 512], i8, tag="wt8", name="wt8")
                        nc.sync.dma_start(out=wt8[:], in_=g8[rt * P:(rt + 1) * P, c0 + ct * 512:c0 + (ct + 1) * 512])
                        wtf = dqp.tile([P, 512], bf16, tag="wtf", name="wtf")
                        nc.vector.tensor_scalar_mul(wtf[:], wt8[:], sclw[:])
                        nc.sync.dma_start(out=gf[rt * P:(rt + 1) * P, c0 + ct * 512:c0 + (ct + 1) * 512], in_=wtf[:])

        with tc.tile_pool(name="const", bufs=1) as constp:
            ones = constp.tile([P, 1], f32r)
            nc.vector.memset(ones[:].bitcast(f32), 1.0)
            onesb = constp.tile([P, 1], bf16)
            nc.vector.memset(onesb[:], 1.0)
            khm = constp.tile([WS, T], bf16)
            kwm = constp.tile([WS, T], bf16)
            nc.gpsimd.dma_start(out=khm[:], in_=khm_d[:])
            nc.gpsimd.dma_start(out=kwm[:], in_=kwm_d[:])
            rhT8 = constp.tile([P, WS, WS], bf16)
            rwT8 = constp.tile([P, WS, WS], bf16)
            nc.gpsimd.dma_start(out=rhT8[:], in_=rhT8_d[:])
            nc.gpsimd.dma_start(out=rwT8[:], in_=rwT8_d[:])

            # ---- LN stats along the partition (feature) axis via ones-matmul
            def ln_stats(src_tiles, rstd, nmr, use_bf16):
                sq_dt = bf16 if use_bf16 else f32r
                ones_t = onesb if use_bf16 else ones
                mm = (lambda a: a) if use_bf16 else r
                with tc.tile_pool(name="sq", bufs=3) as sqp, \
                     tc.tile_pool(name="pstat", bufs=1, space="PSUM") as pstat, \
                     tc.tile_pool(name="stat", bufs=1) as statp:
                    ssum = statp.tile([1, TOKP], f32, tag="ssum", name="ssum")[:]
                    ssq = statp.tile([1, TOKP], f32, tag="ssq", name="ssq")[:]
                    for t in range(NT):
                        ps = pstat.tile([1, 512], f32, tag="ps")
                        ps2 = pstat.tile([1, 512], f32, tag="ps2")
                        for k in range(KD):
                            sq = sqp.tile([P, 512], sq_dt)
                            nc.scalar.activation(sq[:], src_tiles[k][:, ts(t, 512)], AF.Square)
                            nc.tensor.matmul(ps[:], lhsT=mm(ones_t[:]),
                                             rhs=mm(src_tiles[k][:, ts(t, 512)]),
                                             start=(k == 0), stop=(k == KD - 1))
                            nc.tensor.matmul(ps2[:], lhsT=mm(ones_t[:]), rhs=mm(sq[:]),
                                             start=(k == 0), stop=(k == KD - 1))
                        nc.vector.tensor_copy(ssum[:, ts(t, 512)], ps[:])
                        nc.vector.tensor_copy(ssq[:, ts(t, 512)], ps2[:])
                    # mean=ssum/D; msq=ssq/D; var=msq-mean^2; rstd=1/sqrt(var+eps)
                    nc.vector.tensor_scalar_mul(ssum, ssum, 1.0 / DIM)
                    nc.vector.tensor_scalar_mul(ssq, ssq, 1.0 / DIM)
                    tmp = statp.tile([1, TOKP], f32, tag="tmp", name="tmp")[:]
                    rstd1r = statp.tile([1, TOKP], f32, tag="rstd1r", name="rstd1r")[:]
                    nc.vector.tensor_mul(tmp, ssum, ssum)
                    nc.vector.tensor_sub(ssq, ssq, tmp)
                    nc.vector.tensor_scalar_add(ssq, ssq, float(EPS))
                    nc.scalar.activation(tmp, ssq, AF.Sqrt)
                    nc.vector.reciprocal(rstd1r, tmp)
                    nc.vector.tensor_mul(tmp, ssum, rstd1r)
                    nc.sync.dma_start(out=ln_scr[0:1, :], in_=rstd1r)
                    nc.sync.dma_start(out=ln_scr[1:2, :], in_=tmp)
                    nc.sync.dma_start(out=rstd[:], in_=ln_scr[0:1, :].to_broadcast((P, TOKP)))
                    nc.sync.dma_start(out=nmr[:], in_=ln_scr[1:2, :].to_broadcast((P, TOKP)))


            # ================= phase 1+2: LN1 + QKV + V =================
            with tc.tile_pool(name="xT", bufs=1) as xTp:
                xT = []
                with tc.tile_pool(name="x8", bufs=1) as x8p, \
                     tc.tile_pool(name="xsc", bufs=1) as xscp:
                    scl = xscp.tile([P, 1], f32, tag="scl")
                    nc.sync.dma_start(out=scl[:], in_=xscl_d[:])
                    for k in range(KD):
                        t8 = x8p.tile([P, TOKP], mybir.dt.int8, tag=f"x8{k}",
                                      name=f"x8t{k}")
                        nc.vector.memset(t8[:].bitcast(f32), 0.0)
                        for sl, off, ri, rj in _SLOT_GEOM:
                            n_ = ri * rj
                            src = x8_d[k * P:(k + 1) * P, off:off + n_]
                            if rj == WS:
                                nc.sync.dma_start(out=t8[:, sl * T:sl * T + n_], in_=src)
                            else:
                                nc.sync.dma_start(
                                    out=t8[:, sl * T:(sl + 1) * T].rearrange(
                                        "p (i j) -> p i j", i=WS)[:, 0:ri, 0:rj],
                                    in_=src.rearrange("p (i j) -> p i j", i=ri))
                        t_ = xTp.tile([P, TOKP], bf16, tag=f"xT{k}", name=f"xT{k}")
                        nc.vector.tensor_scalar_mul(t_[:], t8[:], scl[:])
                        xT.append(t_)

                with tc.tile_pool(name="yT", bufs=1) as yTp, \
                     tc.tile_pool(name="lnvec", bufs=1) as lnv:
                    rstd1 = lnv.tile([P, TOKP], f32, tag="rstd1")
                    nmr1 = lnv.tile([P, TOKP], f32, tag="nmr1")
                    ln_stats(xT, rstd1, nmr1, use_bf16=True)
                    yT = []
                    for k in range(KD):
                        t_ = yTp.tile([P, TOKP], bf16, tag=f"yT{k}", name=f"yT{k}")
                        nc.vector.tensor_mul(t_[:], xT[k][:], rstd1[:])
                        nc.vector.tensor_sub(t_[:], t_[:], nmr1[:])
                        yT.append(t_)

                    with tc.tile_pool(name="wqk", bufs=12) as wp, \
                         tc.tile_pool(name="qkps", bufs=1, space="PSUM") as qkps, \
                         tc.tile_pool(name="ev", bufs=6) as evp, \
                         tc.tile_pool(name="bias", bufs=1) as biasp:
                        bqall = biasp.tile([P, 16], f32, tag="bqall")
                        nc.sync.dma_start(out=bqall[:],
                                          in_=bqkv_d[0:2 * DIM, :].rearrange("(m p) one -> p (m one)", p=P))
                        for m in range(16):
                            pss = [qkps.tile([P, 512], f32, tag=f"qk{t}", name=f"qkps{t}") for t in range(NT)]
                            wt = wp.tile([P, KD, P], bf16)
                            nc.sync.dma_start(
                                out=wt[:],
                                in_=wqkv_g[:, m * P:(m + 1) * P].rearrange("(g p) c -> p g c", p=P))
                            for k in range(KD):
                                for t in range(NT):
                                    nc.tensor.matmul(pss[t][:], lhsT=wt[:, k, :],
                                                     rhs=yT[k][:, ts(t, 512)],
                                                     start=(k == 0), stop=(k == KD - 1))
                            for t in range(NT):
                                ev = evp.tile([P, 512], bf16)
                                nc.vector.tensor_scalar_add(ev[:], pss[t][:], bqall[:, m:m + 1])
                                nc.sync.dma_start(out=qk_scr[m * P:(m + 1) * P, ts(t, 512)], in_=ev[:])

                        wv = []
                        for k in range(KD):
                            wvt = wp.tile([P, DIM], bf16, tag=f"wv{k}", name=f"wv{k}", bufs=1)
                            nc.sync.dma_start(out=wvt[:], in_=wqkv_g[k * P:(k + 1) * P, 2 * DIM:3 * DIM])
                            wv.append(wvt)
                        bvrow = biasp.tile([P, DIM], f32, tag="bvrow", bufs=1)
                        nc.sync.dma_start(out=bvrow[:], in_=bqkv_d[2 * DIM:3 * DIM, :].rearrange("d one -> one d").to_broadcast((P, DIM)))
                        for tk in range(TOKP // P):
                            psv = [qkps.tile([P, 512], f32, tag=f"v{j}", name=f"psv{j}") for j in range(2)]
                            for k in range(KD):
                                for j in range(2):
                                    nc.tensor.matmul(psv[j][:], lhsT=yT[k][:, ts(tk, P)],
                                                     rhs=wv[k][:, ts(j, 512)],
                                                     start=(k == 0), stop=(k == KD - 1))
                            for j in range(2):
                                ev = evp.tile([P, 512], bf16)
                                nc.vector.tensor_add(ev[:], psv[j][:], bvrow[:, ts(j, 512)])
                                nc.sync.dma_start(out=v_scr[tk * P:(tk + 1) * P, ts(j, 512)], in_=ev[:])

                # ============ phase 2.5: rel-pos bias from q on device ==
                # Load q fully into SBUF [p, g, w, i, j]; build an (i,j)-
                # transposed copy [p, g, w, j, i] with a strided DVE copy so
                # both the per-i and per-j matmul slices read contiguously.
                with tc.tile_pool(name="qall", bufs=1) as qap, \
                     tc.tile_pool(name="rstage", bufs=2) as rsgp, \
                     tc.tile_pool(name="relps", bufs=4, space="PSUM") as rpp:
                    q_all = qap.tile([P, KD, WPC, WS, WS], bf16, tag="qa")
                    qT_all = qap.tile([P, KD, WPC, WS, WS], bf16, tag="qt")
                    for g in range(KD):
                        nc.sync.dma_start(
                            out=q_all[:, g],
                            in_=qk_scr[g * P:(g + 1) * P, 0:TOK].rearrange(
                                "p (w a b) -> p w a b", a=WS, b=WS))
                        nc.vector.tensor_copy(
                            qT_all[:, g],
                            q_all[:, g].rearrange("p w a b -> p w b a"))
                    for i in range(WS):
                        stg = rsgp.tile([WS, NH, WPC, WS], bf16, tag="stgh")
                        for h in range(NH):
                            g, bp = h // 2, 64 * (h % 2)
                            psr = rpp.tile([WS, WPC * WS], f32, tag="psr")
                            nc.tensor.matmul(psr[:], lhsT=rhT8[bp:bp + 64, i, :],
                                             rhs=q_all[bp:bp + 64, g, :, i, :],
                                             start=True, stop=True)
                            nc.vector.tensor_copy(stg[:, h, :, :], psr[:])
                        nc.sync.dma_start(out=relh_scr[:, :, :, i, :], in_=stg[:])
                    for j in range(WS):
                        stg = rsgp.tile([WS, NH, WPC, WS], bf16, tag="stgw")
                        for h in range(NH):
                            g, bp = h // 2, 64 * (h % 2)
                            psr = rpp.tile([WS, WPC * WS], f32, tag="psr")
                            nc.tensor.matmul(psr[:], lhsT=rwT8[bp:bp + 64, j, :],
                                             rhs=qT_all[bp:bp + 64, g, :, j, :],
                                             start=True, stop=True)
                            nc.vector.tensor_copy(stg[:, h, :, :], psr[:])
                        nc.sync.dma_start(out=relw_scr[:, :, :, j, :], in_=stg[:])

                # ================= phase 3: windowed attention ==========
                with tc.tile_pool(name="wload", bufs=3) as wl, \
                     tc.tile_pool(name="relload", bufs=3) as rl, \
                     tc.tile_pool(name="vload", bufs=3) as vl, \
                     tc.tile_pool(name="expt", bufs=6) as ep, \
                     tc.tile_pool(name="rsp", bufs=8) as rsp, \
                     tc.tile_pool(name="aout", bufs=6) as aop, \
                     tc.tile_pool(name="lps", bufs=2, space="PSUM") as lps, \
                     tc.tile_pool(name="sps", bufs=2, space="PSUM") as sps, \
                     tc.tile_pool(name="ops", bufs=2, space="PSUM") as ops:
                    # software pipeline over (w, h): logits for iteration i+1
                    # are issued on the PE before iteration i's consumers
                    # (ssm/ov), so the in-order PE never idles waiting for
                    # the Act-engine exp of the current iteration.
                    wins = {}

                    def load_window(w):
                        kw_t = wl.tile([P, KD, T], bf16, tag="kw", name="kw_t")
                        qw_t = wl.tile([P, KD, T], bf16, tag="qw", name="qw_t")
                        nc.gpsimd.dma_start(
                            out=kw_t[:],
                            in_=qk_scr[DIM:2 * DIM, w * T:(w + 1) * T].rearrange("(g p) c -> p g c", p=P))
                        nc.gpsimd.dma_start(
                            out=qw_t[:],
                            in_=qk_scr[0:DIM, w * T:(w + 1) * T].rearrange("(g p) c -> p g c", p=P))
                        relh_t = rl.tile([WS, NH, WS, WS], bf16, tag="rh", name="relh_t")
                        relw_t = rl.tile([WS, NH, WS, WS], bf16, tag="rw", name="relw_t")
                        nc.gpsimd.dma_start(out=relh_t[:], in_=relh_scr[:, :, w, :, :])
                        nc.gpsimd.dma_start(out=relw_t[:], in_=relw_scr[:, :, w, :, :])
                        vw0 = vl.tile([P, DIM], bf16, tag="v0", name="vw0")
                        vw1 = vl.tile([68, DIM], bf16, tag="v1", name="vw1")
                        nc.gpsimd.dma_start(out=vw0[:], in_=v_scr[w * T:w * T + P, :])
                        nc.gpsimd.dma_start(out=vw1[:], in_=v_scr[w * T + P:(w + 1) * T, :])
                        return kw_t, qw_t, relh_t, relw_t, vw0, vw1

                    def logits_stage(w, h):
                        kw_t, qw_t, relh_t, relw_t, vw0, vw1 = wins[w]
                        g, bp = h // 2, 64 * (h % 2)
                        lA = lps.tile([P, T], f32, tag="lA", name="lA")
                        lB = lps.tile([68, T], f32, tag="lB", name="lB")
                        qs = qw_t[bp:bp + 64, g, :]
                        nc.tensor.matmul(lA[:], lhsT=kw_t[bp:bp + 64, g, 0:P], rhs=qs,
                                         start=True, stop=False)
                        nc.tensor.matmul(lA[:], lhsT=khm[:, 0:P], rhs=relh_t[:, h],
                                         start=False, stop=False)
                        nc.tensor.matmul(lA[:], lhsT=kwm[:, 0:P],
                                         rhs=relw_t[:, h].rearrange("kj a b -> kj b a"),
                                         start=False, stop=True)
                        nc.tensor.matmul(lB[:], lhsT=kw_t[bp:bp + 64, g, P:T], rhs=qs,
                                         start=True, stop=False)
                        nc.tensor.matmul(lB[:], lhsT=khm[:, P:T], rhs=relh_t[:, h],
                                         start=False, stop=False)
                        nc.tensor.matmul(lB[:], lhsT=kwm[:, P:T],
                                         rhs=relw_t[:, h].rearrange("kj a b -> kj b a"),
                                         start=False, stop=True)
                        return lA, lB

                    def exp_stage(lA, lB):
                        eA = ep.tile([P, T], bf16, tag="eA", name="eA")
                        eB = ep.tile([68, T], bf16, tag="eB", name="eB")
                        nc.scalar.activation(eA[:], lA[:], AF.Exp)
                        nc.scalar.activation(eB[:], lB[:], AF.Exp)
                        return eA, eB

                    def consume_stage(w, h, eA, eB):
                        kw_t, qw_t, relh_t, relw_t, vw0, vw1 = wins[w]
                        ssm = sps.tile([1, T], f32, tag="ssm", name="ssm")
                        nc.tensor.matmul(ssm[:], lhsT=onesb[:], rhs=eA[:],
                                         start=True, stop=False)
                        nc.tensor.matmul(ssm[:], lhsT=onesb[0:68, :], rhs=eB[:],
                                         start=False, stop=True)
                        ov = ops.tile([64, T], f32, tag="ov", name="ov")
                        nc.tensor.matmul(ov[:], lhsT=vw0[:, h * HD:(h + 1) * HD], rhs=eA[:],
                                         start=True, stop=False)
                        nc.tensor.matmul(ov[:], lhsT=vw1[:, h * HD:(h + 1) * HD], rhs=eB[:],
                                         start=False, stop=True)
                        rs = rsp.tile([1, T], f32, tag="rs", name="rs")
                        nc.vector.reciprocal(rs[:], ssm[:])
                        rsP = rsp.tile([64, T], f32, tag="rsP", name="rsP")
                        nc.sync.dma_start(out=rs_scr[h:h + 1, :], in_=rs[:])
                        nc.sync.dma_start(out=rsP[:], in_=rs_scr[h:h + 1, :].to_broadcast((64, T)))
                        ao = aop.tile([64, T], bf16, tag="ao", name="ao")
                        nc.vector.tensor_mul(ao[:], ov[:], rsP[:])
                        nc.sync.dma_start(out=attn_scr[h * HD:(h + 1) * HD, w * T:(w + 1) * T],
                                          in_=ao[:])

                    items = [(w, h) for w in range(WPC) for h in range(NH)]
                    pending = None
                    for it, (w, h) in enumerate(items):
                        if h == 0:
                            wins[w] = load_window(w)
                        lA, lB = logits_stage(w, h)
                        eA, eB = exp_stage(lA, lB)
                        if pending is not None:
                            consume_stage(*pending)
                        pending = (w, h, eA, eB)
                    consume_stage(*pending)

                with tc.tile_pool(name="xres", bufs=1) as xrp:
                    xres = [xrp.tile([P, TOKP], bf16, tag=f"xr{k}", name=f"xres{k}")
                            for k in range(KD)]
                    # ================= phase 4: proj + residual =============
                    with tc.tile_pool(name="wpj", bufs=1) as wp2, \
                         tc.tile_pool(name="pjps", bufs=1, space="PSUM") as pjps, \
                         tc.tile_pool(name="aload", bufs=6) as alp, \
                         tc.tile_pool(name="bias2", bufs=1) as biasp2:
                        wpj = []
                        for k in range(KD):
                            row = []
                            for m in range(KD):
                                wt = wp2.tile([P, P], bf16, tag=f"pj{k}_{m}", name=f"wpj{k}_{m}")
                                nc.sync.dma_start(out=wt[:], in_=wproj_g[k * P:(k + 1) * P, m * P:(m + 1) * P])
                                row.append(wt)
                            wpj.append(row)
                        bpjs = []
                        for m in range(KD):
                            bt = biasp2.tile([P, 1], f32, tag=f"bpj{m}", name=f"bpj{m}")
                            nc.sync.dma_start(out=bt[:], in_=bproj_d[m * P:(m + 1) * P, :])
                            bpjs.append(bt)
                        for t in range(NT):
                            pss = [pjps.tile([P, 512], f32, tag=f"pj{m}", name=f"pjps{m}") for m in range(KD)]
                            for k in range(KD):
                                at = alp.tile([P, 512], bf16, tag="at")
                                nc.sync.dma_start(out=at[:], in_=attn_scr[k * P:(k + 1) * P, ts(t, 512)])
                                for m in range(KD):
                                    nc.tensor.matmul(pss[m][:], lhsT=wpj[k][m][:], rhs=at[:],
                                                     start=(k == 0), stop=(k == KD - 1))
                            for m in range(KD):
                                nc.vector.tensor_scalar_add(xres[m][:, ts(t, 512)], pss[m][:], bpjs[m][:])
                                nc.vector.tensor_add(xres[m][:, ts(t, 512)],
                                                     xres[m][:, ts(t, 512)], xT[m][:, ts(t, 512)])

                    # ================= phase 5: LN2 + MLP =================
                    with tc.tile_pool(name="lnvec2", bufs=1) as lnv2, \
                         tc.tile_pool(name="rmaxp", bufs=1) as rmp:
                        rstd2 = lnv2.tile([P, TOKP], f32, tag="rstd2")
                        nmr2 = lnv2.tile([P, TOKP], f32, tag="nmr2")
                        rmax = [rmp.tile([P, 1], f32, tag=f"rmax{m}", name=f"rmax{m}")
                                for m in range(KD)]
                        rtmp = rmp.tile([P, 1], f32, tag="rtmp")
                        ln_stats(xres, rstd2, nmr2, use_bf16=True)

                        with tc.tile_pool(name="xn", bufs=2) as xnp, \
                             tc.tile_pool(name="z1", bufs=34) as z1p, \
                             tc.tile_pool(name="wmlp", bufs=6) as wmp, \
                             tc.tile_pool(name="z1ps", bufs=3, space="PSUM") as z1ps, \
                             tc.tile_pool(name="z2ps", bufs=1, space="PSUM") as z2ps, \
                             tc.tile_pool(name="bias3", bufs=1) as biasp3, \
                             tc.tile_pool(name="outp", bufs=6) as outp:
                            b2ts = []
                            for m in range(KD):
                                bt2 = biasp3.tile([P, 1], f32, tag=f"b2{m}", name=f"b2t{m}")
                                nc.sync.dma_start(out=bt2[:], in_=b2_d[m * P:(m + 1) * P, :])
                                b2ts.append(bt2)
                            b1all = biasp3.tile([P, DFF // P], f32, tag="b1all")
                            nc.sync.dma_start(out=b1all[:],
                                              in_=b1_d[:].rearrange("(d p) one -> p (d one)", p=P))
                            for t in range(NT):
                                xnt = xnp.tile([P, KD, 512], bf16, tag="xnt")
                                for k in range(KD):
                                    nc.vector.tensor_mul(xnt[:, k, :], xres[k][:, ts(t, 512)],
                                                         rstd2[:, ts(t, 512)])
                                    nc.vector.tensor_sub(xnt[:, k, :], xnt[:, k, :],
                                                         nmr2[:, ts(t, 512)])
                                z1s = []
                                for d in range(DFF // P):
                                    psz = z1ps.tile([P, 512], f32, tag="psz")
                                    w1c = wmp.tile([P, KD, P], bf16, tag="w1c")
                                    nc.sync.dma_start(
                                        out=w1c[:],
                                        in_=w1_g[:, d * P:(d + 1) * P].rearrange("(g p) c -> p g c", p=P))
                                    for k in range(KD):
                                        nc.tensor.matmul(psz[:], lhsT=w1c[:, k, :], rhs=xnt[:, k, :],
                                                         start=(k == 0), stop=(k == KD - 1))
                                    z1 = z1p.tile([P, 512], bf16, tag="z1", name=f"z1_{t}_{d}")
                                    nc.scalar.activation(z1[:], psz[:], AF.Gelu, bias=b1all[:, d:d + 1])
                                    z1s.append(z1)
                                for mg in range(2):
                                    psos = [z2ps.tile([P, 512], f32, tag=f"z2{j}", name=f"z2ps{j}") for j in range(4)]
                                    for d in range(DFF // P):
                                        w2c = wmp.tile([P, 512], bf16, tag="w2c")
                                        nc.sync.dma_start(out=w2c[:], in_=w2_g[d * P:(d + 1) * P, mg * 512:(mg + 1) * 512])
                                        for j in range(4):
                                            nc.tensor.matmul(psos[j][:], lhsT=w2c[:, ts(j, P)], rhs=z1s[d][:],
                                                             start=(d == 0), stop=(d == DFF // P - 1))
                                    for j in range(4):
                                        m = mg * 4 + j
                                        otf = outp.tile([P, 512], f32, tag="otf", name="otf")
                                        ot = outp.tile([P, 512], bf16, tag="ot", name="ot")
                                        nc.vector.tensor_scalar_add(otf[:], psos[j][:], b2ts[m][:])
                                        nc.vector.tensor_add(otf[:], otf[:], xres[m][:, ts(t, 512)])
                                        nc.vector.tensor_sub(ot[:], otf[:], xT[m][:, ts(t, 512)])
                                        if t == 0:
                                            nc.vector.tensor_reduce(
                                                rmax[m][:], ot[:], axis=mybir.AxisListType.XYZW,
                                                op=mybir.AluOpType.max, apply_absolute_value=True)
                                        else:
                                            nc.vector.tensor_reduce(
                                                rtmp[:], ot[:], axis=mybir.AxisListType.XYZW,
                                                op=mybir.AluOpType.max, apply_absolute_value=True)
                                            nc.vector.tensor_max(rmax[m][:], rmax[m][:], rtmp[:])
                                        nc.sync.dma_start(out=dlt_scr[m * P:(m + 1) * P, ts(t, 512)], in_=ot[:])
                        # ---- int8 delta quantization with per-feature scales
                        with tc.tile_pool(name="qout", bufs=6) as qop, \
                             tc.tile_pool(name="rinvp", bufs=1) as rip:
                            for m in range(KD):
                                nc.vector.tensor_scalar_max(rmax[m][:], rmax[m][:], 1e-20)
                                sc_o = rip.tile([P, 1], f32, tag=f"sco{m}", name=f"sco{m}")
                                nc.vector.tensor_scalar_mul(sc_o[:], rmax[m][:], 1.0 / 127.0)
                                nc.sync.dma_start(out=oscl_d[m * P:(m + 1) * P, :], in_=sc_o[:])
                                rinv = rip.tile([P, 1], f32, tag=f"rin{m}", name=f"rin{m}")
                                nc.vector.reciprocal(rinv[:], sc_o[:])
                                o8f = qop.tile([P, TOKP], mybir.dt.int8, tag="o8f",
                                               name="o8f", bufs=2)
                                for t in range(NT):
                                    blk = qop.tile([P, 512], bf16, tag="blk", name="blk")
                                    nc.sync.dma_start(out=blk[:], in_=dlt_scr[m * P:(m + 1) * P, ts(t, 512)])
                                    nc.vector.tensor_scalar_mul(o8f[:, ts(t, 512)], blk[:], rinv[:])
                                for sl, off, ri, rj in _SLOT_GEOM:
                                    n_ = ri * rj
                                    if rj == WS:
                                        nc.sync.dma_start(
                                            out=outT_d[m * P:(m + 1) * P, off:off + n_],
                                            in_=o8f[:, sl * T:sl * T + n_])
                                    else:
                                        nc.sync.dma_start(
                                            out=outT_d[m * P:(m + 1) * P, off:off + n_].rearrange(
                                                "p (i j) -> p i j", i=ri),
                                            in_=o8f[:, sl * T:(sl + 1) * T].rearrange(
                                                "p (i j) -> p i j", i=WS)[:, 0:ri, 0:rj])
    nc.compile()
    return nc


def kernel(**inputs):
    from concourse.bass_utils import run_bass_kernel_spmd

    if "nc" not in _CACHE:
        _CACHE["nc"] = _build()
    nc = _CACHE["nc"]
    in_maps = _hostprep(**inputs)
    res = run_bass_kernel_spmd(nc, in_maps, list(range(8)))
    delta = np.zeros((B, 64, 64, DIM), np.float32)
    for c in range(8):
        o = res.results[c]["outT"].astype(np.float32) * res.results[c]["oscl"]
        for sl, off, ri, rj in _SLOT_GEOM:
            wspec = _WIN_ASSIGN[c][sl]
            if wspec is None:
                continue
            b_, wi, wj = wspec
            delta[b_, 14 * wi:14 * wi + ri, 14 * wj:14 * wj + rj, :] = \
                o[:, off:off + ri * rj].T.reshape(ri, rj, DIM)
    return (np.asarray(inputs["x"], np.float32) + delta).astype(np.float32)

